# revision 2
# baseline (speedup 1.0000x reference)
"""Multi-head self-attention 2D Bass kernel v5 for Trainium2.

Sharding (8 cores): core i handles batch b = i//2 and HEAD-GROUP g = i%2
  (heads 4g..4g+3), all N=2304 queries.  Per-core: Q/K/V for its 4 heads
  only (no K/V redundancy vs batch-sharding), attention, and a PARTIAL
  projection y_g = Wp[:, g] @ out_g; the host sums the two fp16 partials
  per batch.

Structure (all matmul operands fp16, PSUM fp32, simple in-order
emission -- measured fastest on HW; interleaved/software-pipelined
variants regressed):
  phase1: V (18 m-tiles), then K,Q for pair 0 and pair 1.
  phase2: 8 main blocks (pair t x 512-query chunk c; per m-tile:
    row-packed S pair -> one wide exp [128,1024] (scale folded) ->
    AV pair accumulating with ones-row denominators), then ONE joint
    tail block covering the last 256 queries of BOTH pairs (4 S
    quadrants in bank-separated PSUM, still one wide exp per m).
  Per-block finalize: reciprocal straight from the PSUM denominator
  row, ones-matmul broadcast, normalize, projection slice into oy
  (pair 0 copy / pair 1 add); y cols 0:2048 DMA after the last main
  block, tail cols after the tail block.
"""

import numpy as np

B = 4
C = 512
HH = 48
WW = 48
N = HH * WW          # 2304
HEADS = 8
HG = 4               # heads per core
GC = HG * 64         # 256 channels per group
D = 64
SCALE = float(D) ** -0.5
NCORES = 8

_CACHE: dict = {}


def _build_module(loop_n=None):
    import concourse.mybir as mybir
    import concourse.tile as tile
    from concourse import bacc

    FP16 = mybir.dt.float16
    FP32 = mybir.dt.float32
    AF = mybir.ActivationFunctionType

    nc = bacc.Bacc("TRN2", target_bir_lowering=False, debug=False)
    xf_d = nc.dram_tensor("xf", [C, N], FP16, kind="ExternalInput")
    wqkv = nc.dram_tensor("wqkv", [C, 3 * GC], FP16, kind="ExternalInput")
    wproj = nc.dram_tensor("wproj", [GC, C], FP16, kind="ExternalInput")
    y = nc.dram_tensor("y", [C, N], FP16, kind="ExternalOutput")

    CT = C // 128     # 4 channel tiles of x
    MT = N // 128     # 18 key tiles
    NT = 2048         # tail start

    with tile.TileContext(nc) as tc:
        with (
            tc.tile_pool(name="consts", bufs=1) as cpool,
            tc.tile_pool(name="wts", bufs=1) as wpool,
            tc.tile_pool(name="xin", bufs=1) as xpool,
            tc.tile_pool(name="qkv", bufs=1) as qkpool,
            tc.tile_pool(name="keep", bufs=1) as keep,
        ):
            ones1 = cpool.tile([1, 64], FP16, name="ones1", tag="ones1")
            nc.vector.memset(ones1[:], 1.0)

            wt = []
            for t in range(CT):
                w = wpool.tile([128, 3 * GC], FP16, name=f"w{t}", tag=f"w{t}")
                nc.sync.dma_start(w[:], wqkv.ap()[128 * t : 128 * (t + 1), :])
                wt.append(w)
            wp = []
            for t in range(2):
                p = wpool.tile([128, C], FP16, name=f"wp{t}", tag=f"wp{t}")
                nc.sync.dma_start(p[:], wproj.ap()[128 * t : 128 * (t + 1), :])
                wp.append(p)

            qsb = [qkpool.tile([128, N], FP16, name=f"q{t}", tag=f"q{t}") for t in range(2)]
            ksb = [qkpool.tile([128, N], FP16, name=f"k{t}", tag=f"k{t}") for t in range(2)]
            vsb = [qkpool.tile([128, 4 * 65], FP16, name=f"v{m}", tag=f"v{m}") for m in range(MT)]
            avsb = [keep.tile([128, N], FP16, name=f"av{t}", tag=f"av{t}") for t in range(2)]
            oa = [keep.tile([128, N], FP16, name=f"oa{t}", tag=f"oa{t}") for t in range(2)]
            oy = [keep.tile([128, N], FP16, name=f"oy{ct}", tag=f"oy{ct}") for ct in range(CT)]
            rec = [
                [keep.tile([1, N], FP16, name=f"rec{t}_{h}", tag=f"rec{t}_{h}") for h in range(2)]
                for t in range(2)
            ]

            xf = []
            for t in range(CT):
                xt = xpool.tile([128, N], FP16, name=f"x{t}", tag=f"x{t}")
                nc.scalar.dma_start(
                    xt[:, 0 : N // 2], xf_d.ap()[128 * t : 128 * (t + 1), 0 : N // 2]
                )
                xf.append(xt)
            for t in range(CT):
                nc.scalar.dma_start(
                    xf[t][:, N // 2 : N],
                    xf_d.ap()[128 * t : 128 * (t + 1), N // 2 : N],
                )

            def _run(spool, avps, flex, epool):
                # ---- phase1: V, then K/Q for both pairs ----
                for m in range(MT):
                    psv = flex.tile([128, 512], FP32, name="psv", tag="flex")
                    for kt in range(CT):
                        nc.tensor.matmul(
                            psv[:, 0:GC],
                            lhsT=xf[kt][:, 128 * m : 128 * (m + 1)],
                            rhs=wt[kt][:, 2 * GC : 3 * GC],
                            start=(kt == 0),
                            stop=(kt == CT - 1),
                        )
                    v3 = vsb[m][:].rearrange("p (h w) -> p h w", h=HG)
                    nc.vector.memset(v3[:, :, 64:65], 1.0)
                    nc.vector.tensor_copy(
                        v3[:, :, 0:64],
                        psv[:, 0:GC].rearrange("p (h w) -> p h w", h=HG),
                    )

                for t in range(2):
                    for which, col, dst in (
                        ("k", GC + 128 * t, ksb[t]),
                        ("q", 128 * t, qsb[t]),
                    ):
                        for (n0, nl) in [(0, 1024), (1024, 1024), (2048, 256)]:
                            ps = spool.tile([128, 1024], FP32, name="pkq", tag="s")
                            for kt in range(CT):
                                for c0 in range(0, nl, 512):
                                    cl = min(512, nl - c0)
                                    nc.tensor.matmul(
                                        ps[:, c0 : c0 + cl],
                                        lhsT=wt[kt][:, col : col + 128],
                                        rhs=xf[kt][:, n0 + c0 : n0 + c0 + cl],
                                        start=(kt == 0),
                                        stop=(kt == CT - 1),
                                    )
                            nc.vector.tensor_copy(dst[:, n0 : n0 + nl], ps[:, 0:nl])

                # ---- phase2 helpers ----
                def normalize_quadrant(t, half, c0, cl, av, q0):
                    """recip from PSUM row 64 of av[:, q0:q0+cl]; bcast; normalize oa."""
                    with nc.allow_low_precision(reason="softmax recip fp16"):
                        nc.vector.reciprocal(
                            rec[t][half][:, c0 : c0 + cl], av[64:65, q0 : q0 + cl]
                        )
                    bc = flex.tile([128, 512], FP32, name="bc", tag="flex")
                    nc.tensor.matmul(
                        bc[0:64, 0:cl], lhsT=ones1[:],
                        rhs=rec[t][half][:, c0 : c0 + cl],
                        start=True, stop=True,
                    )
                    nc.vector.tensor_mul(
                        oa[t][64 * half : 64 * half + 64, c0 : c0 + cl],
                        avsb[t][64 * half : 64 * half + 64, c0 : c0 + cl],
                        bc[0:64, 0:cl],
                    )

                # ---- 8 main blocks ----
                for t in range(2):
                    for c0 in range(0, NT, 512):
                        avA = avps.tile([65, 512], FP32, name="avA", tag="avA")
                        avB = avps.tile([65, 512], FP32, name="avB", tag="avB")
                        for m in range(MT):
                            ms = slice(128 * m, 128 * (m + 1))
                            sp = spool.tile([128, 1024], FP32, name="s", tag="s")
                            nc.tensor.matmul(
                                sp[:, 0:512], lhsT=ksb[t][0:64, ms],
                                rhs=qsb[t][0:64, c0 : c0 + 512],
                                start=True, stop=True, tile_position=(0, 0),
                            )
                            nc.tensor.matmul(
                                sp[:, 512:1024], lhsT=ksb[t][64:128, ms],
                                rhs=qsb[t][64:128, c0 : c0 + 512],
                                start=True, stop=True, tile_position=(64, 0),
                            )
                            es = epool.tile([128, 1024], FP16, name="es", tag="es")
                            nc.scalar.activation(es[:], sp[:], AF.Exp, scale=SCALE)
                            nc.tensor.matmul(
                                avA[:], lhsT=vsb[m][:, 130 * t : 130 * t + 65],
                                rhs=es[:, 0:512],
                                start=(m == 0), stop=(m == MT - 1),
                            )
                            nc.tensor.matmul(
                                avB[:], lhsT=vsb[m][:, 130 * t + 65 : 130 * t + 130],
                                rhs=es[:, 512:1024],
                                start=(m == 0), stop=(m == MT - 1),
                            )
                        nc.vector.tensor_copy(avsb[t][0:64, c0 : c0 + 512], avA[0:64, :])
                        nc.vector.tensor_copy(avsb[t][64:128, c0 : c0 + 512], avB[0:64, :])
                        normalize_quadrant(t, 0, c0, 512, avA, 0)
                        normalize_quadrant(t, 1, c0, 512, avB, 0)
                        for ct in range(CT):
                            py = flex.tile([128, 512], FP32, name="py", tag="flex")
                            nc.tensor.matmul(
                                py[:],
                                lhsT=wp[t][:, 128 * ct : 128 * (ct + 1)],
                                rhs=oa[t][:, c0 : c0 + 512],
                                start=True, stop=True,
                            )
                            if t == 0:
                                nc.vector.tensor_copy(oy[ct][:, c0 : c0 + 512], py[:])
                            else:
                                nc.vector.tensor_add(
                                    oy[ct][:, c0 : c0 + 512],
                                    oy[ct][:, c0 : c0 + 512],
                                    py[:],
                                )
                        if t == 1 and c0 + 512 == NT:
                            for ct2 in range(CT):
                                eng = nc.sync if ct2 % 2 == 0 else nc.scalar
                                eng.dma_start(
                                    y.ap()[128 * ct2 : 128 * (ct2 + 1), 0:NT],
                                    oy[ct2][:, 0:NT],
                                )

                # ---- joint tail block: last 256 queries of both pairs ----
                # sp quadrants: p0A [0:256], p1A [256:512] (row group 0);
                #               p0B [512:768], p1B [768:1024] (row group 64).
                avA = avps.tile([65, 512], FP32, name="avA", tag="avA")
                avB = avps.tile([65, 512], FP32, name="avB", tag="avB")
                av2A = flex.tile([128, 512], FP32, name="av2A", tag="flex")
                av2B = flex.tile([128, 512], FP32, name="av2B", tag="flex")
                for m in range(MT):
                    ms = slice(128 * m, 128 * (m + 1))
                    sp = spool.tile([128, 1024], FP32, name="s", tag="s")
                    nc.tensor.matmul(
                        sp[:, 0:256], lhsT=ksb[0][0:64, ms],
                        rhs=qsb[0][0:64, NT:N],
                        start=True, stop=True, tile_position=(0, 0),
                    )
                    nc.tensor.matmul(
                        sp[:, 256:512], lhsT=ksb[1][0:64, ms],
                        rhs=qsb[1][0:64, NT:N],
                        start=True, stop=True, tile_position=(0, 0),
                    )
                    nc.tensor.matmul(
                        sp[:, 512:768], lhsT=ksb[0][64:128, ms],
                        rhs=qsb[0][64:128, NT:N],
                        start=True, stop=True, tile_position=(64, 0),
                    )
                    nc.tensor.matmul(
                        sp[:, 768:1024], lhsT=ksb[1][64:128, ms],
                        rhs=qsb[1][64:128, NT:N],
                        start=True, stop=True, tile_position=(64, 0),
                    )
                    es = epool.tile([128, 1024], FP16, name="es", tag="es")
                    nc.scalar.activation(es[:], sp[:], AF.Exp, scale=SCALE)
                    nc.tensor.matmul(
                        avA[:, 0:256], lhsT=vsb[m][:, 0:65],
                        rhs=es[:, 0:256],
                        start=(m == 0), stop=(m == MT - 1),
                    )
                    nc.tensor.matmul(
                        av2A[0:65, 0:256], lhsT=vsb[m][:, 130:195],
                        rhs=es[:, 256:512],
                        start=(m == 0), stop=(m == MT - 1),
                    )
                    nc.tensor.matmul(
                        avB[:, 0:256], lhsT=vsb[m][:, 65:130],
                        rhs=es[:, 512:768],
                        start=(m == 0), stop=(m == MT - 1),
                    )
                    nc.tensor.matmul(
                        av2B[0:65, 0:256], lhsT=vsb[m][:, 195:260],
                        rhs=es[:, 768:1024],
                        start=(m == 0), stop=(m == MT - 1),
                    )
                nc.vector.tensor_copy(avsb[0][0:64, NT:N], avA[0:64, 0:256])
                nc.vector.tensor_copy(avsb[1][0:64, NT:N], av2A[0:64, 0:256])
                nc.vector.tensor_copy(avsb[0][64:128, NT:N], avB[0:64, 0:256])
                nc.vector.tensor_copy(avsb[1][64:128, NT:N], av2B[0:64, 0:256])
                normalize_quadrant(0, 0, NT, 256, avA, 0)
                normalize_quadrant(1, 0, NT, 256, av2A, 0)
                normalize_quadrant(0, 1, NT, 256, avB, 0)
                normalize_quadrant(1, 1, NT, 256, av2B, 0)
                for ct in range(CT):
                    py = flex.tile([128, 512], FP32, name="py", tag="flex")
                    for t in range(2):
                        nc.tensor.matmul(
                            py[:, 0:256],
                            lhsT=wp[t][:, 128 * ct : 128 * (ct + 1)],
                            rhs=oa[t][:, NT:N],
                            start=(t == 0), stop=(t == 1),
                        )
                    nc.vector.tensor_copy(oy[ct][:, NT:N], py[:, 0:256])
                    eng = nc.sync if ct % 2 == 0 else nc.scalar
                    eng.dma_start(
                        y.ap()[128 * ct : 128 * (ct + 1), NT:N], oy[ct][:, NT:N]
                    )

            with (
                tc.tile_pool(name="sps", bufs=2, space="PSUM") as spool,
                tc.tile_pool(name="avps", bufs=1, space="PSUM") as avps,
                tc.tile_pool(name="flex", bufs=2, space="PSUM") as flex,
                tc.tile_pool(name="esb", bufs=6) as epool,
            ):
                import contextlib
                loop_ctx = tc.For_i(0, loop_n, 1) if loop_n else contextlib.nullcontext()
                with loop_ctx:
                    _run(spool, avps, flex, epool)

    nc.compile()
    return nc


def _get_module():
    if "nc" not in _CACHE:
        _CACHE["nc"] = _build_module()
    return _CACHE["nc"]


def make_in_maps(x, qkv_w, proj_w):
    xr = np.asarray(x, dtype=np.float32).reshape(B, C, N)
    qkv_w = np.asarray(qkv_w)
    proj_w = np.asarray(proj_w)
    in_maps = []
    for i in range(NCORES):
        b, g = divmod(i, 2)
        rows = np.r_[
            g * GC : (g + 1) * GC,
            C + g * GC : C + (g + 1) * GC,
            2 * C + g * GC : 2 * C + (g + 1) * GC,
        ]
        wq = np.ascontiguousarray(qkv_w[rows, :].T).astype(np.float16)
        wpj = np.ascontiguousarray(proj_w[:, g * GC : (g + 1) * GC].T).astype(np.float16)
        xc = np.ascontiguousarray(xr[b]).astype(np.float16)
        in_maps.append({"xf": xc, "wqkv": wq, "wproj": wpj})
    return in_maps


def gather_out(results):
    out = np.empty((B, C, N), np.float32)
    for b in range(B):
        out[b] = results[2 * b]["y"].astype(np.float32) + results[2 * b + 1]["y"].astype(
            np.float32
        )
    return out.reshape(B, C, HH, WW)


def kernel(x, qkv_w, proj_w):
    from concourse import bass_utils

    nc = _get_module()
    in_maps = make_in_maps(x, qkv_w, proj_w)
    res = bass_utils.run_bass_kernel_spmd(
        nc, in_maps, core_ids=list(range(NCORES)), trace=False
    )
    return gather_out(res.results)


# revision 3
# speedup vs baseline: 1.0160x; 1.0160x over previous
"""Multi-head self-attention 2D Bass kernel v5b (V woven into block 0).

Sharding (8 cores): core i handles batch b = i//2 and HEAD-GROUP g = i%2
  (heads 4g..4g+3), all N=2304 queries.  Per-core: Q/K/V for its 4 heads
  only (no K/V redundancy vs batch-sharding), attention, and a PARTIAL
  projection y_g = Wp[:, g] @ out_g; the host sums the two fp16 partials
  per batch.

Structure (all matmul operands fp16, PSUM fp32, simple in-order
emission -- measured fastest on HW; interleaved/software-pipelined
variants regressed):
  phase1: V (18 m-tiles), then K,Q for pair 0 and pair 1.
  phase2: 8 main blocks (pair t x 512-query chunk c; per m-tile:
    row-packed S pair -> one wide exp [128,1024] (scale folded) ->
    AV pair accumulating with ones-row denominators), then ONE joint
    tail block covering the last 256 queries of BOTH pairs (4 S
    quadrants in bank-separated PSUM, still one wide exp per m).
  Per-block finalize: reciprocal straight from the PSUM denominator
  row, ones-matmul broadcast, normalize, projection slice into oy
  (pair 0 copy / pair 1 add); y cols 0:2048 DMA after the last main
  block, tail cols after the tail block.
"""

import numpy as np

B = 4
C = 512
HH = 48
WW = 48
N = HH * WW          # 2304
HEADS = 8
HG = 4               # heads per core
GC = HG * 64         # 256 channels per group
D = 64
SCALE = float(D) ** -0.5
NCORES = 8

_CACHE: dict = {}


def _build_module(loop_n=None):
    import concourse.mybir as mybir
    import concourse.tile as tile
    from concourse import bacc

    FP16 = mybir.dt.float16
    FP32 = mybir.dt.float32
    AF = mybir.ActivationFunctionType

    nc = bacc.Bacc("TRN2", target_bir_lowering=False, debug=False)
    xf_d = nc.dram_tensor("xf", [C, N], FP16, kind="ExternalInput")
    wqkv = nc.dram_tensor("wqkv", [C, 3 * GC], FP16, kind="ExternalInput")
    wproj = nc.dram_tensor("wproj", [GC, C], FP16, kind="ExternalInput")
    y = nc.dram_tensor("y", [C, N], FP16, kind="ExternalOutput")

    CT = C // 128     # 4 channel tiles of x
    MT = N // 128     # 18 key tiles
    NT = 2048         # tail start

    with tile.TileContext(nc) as tc:
        with (
            tc.tile_pool(name="consts", bufs=1) as cpool,
            tc.tile_pool(name="wts", bufs=1) as wpool,
            tc.tile_pool(name="xin", bufs=1) as xpool,
            tc.tile_pool(name="qkv", bufs=1) as qkpool,
            tc.tile_pool(name="keep", bufs=1) as keep,
        ):
            ones1 = cpool.tile([1, 64], FP16, name="ones1", tag="ones1")
            nc.vector.memset(ones1[:], 1.0)

            wt = []
            for t in range(CT):
                w = wpool.tile([128, 3 * GC], FP16, name=f"w{t}", tag=f"w{t}")
                nc.sync.dma_start(w[:], wqkv.ap()[128 * t : 128 * (t + 1), :])
                wt.append(w)
            wp = []
            for t in range(2):
                p = wpool.tile([128, C], FP16, name=f"wp{t}", tag=f"wp{t}")
                nc.sync.dma_start(p[:], wproj.ap()[128 * t : 128 * (t + 1), :])
                wp.append(p)

            qsb = [qkpool.tile([128, N], FP16, name=f"q{t}", tag=f"q{t}") for t in range(2)]
            ksb = [qkpool.tile([128, N], FP16, name=f"k{t}", tag=f"k{t}") for t in range(2)]
            vsb = [qkpool.tile([128, 4 * 65], FP16, name=f"v{m}", tag=f"v{m}") for m in range(MT)]
            avsb = [keep.tile([128, N], FP16, name=f"av{t}", tag=f"av{t}") for t in range(2)]
            oa = [keep.tile([128, N], FP16, name=f"oa{t}", tag=f"oa{t}") for t in range(2)]
            oy = [keep.tile([128, N], FP16, name=f"oy{ct}", tag=f"oy{ct}") for ct in range(CT)]
            rec = [
                [keep.tile([1, N], FP16, name=f"rec{t}_{h}", tag=f"rec{t}_{h}") for h in range(2)]
                for t in range(2)
            ]

            xf = []
            for t in range(CT):
                xt = xpool.tile([128, N], FP16, name=f"x{t}", tag=f"x{t}")
                nc.scalar.dma_start(
                    xt[:, 0 : N // 2], xf_d.ap()[128 * t : 128 * (t + 1), 0 : N // 2]
                )
                xf.append(xt)
            for t in range(CT):
                nc.scalar.dma_start(
                    xf[t][:, N // 2 : N],
                    xf_d.ap()[128 * t : 128 * (t + 1), N // 2 : N],
                )

            def _run(spool, avps, flex, epool):
                def v_group(m):
                    psv = flex.tile([128, 512], FP32, name="psv", tag="flex")
                    for kt in range(CT):
                        nc.tensor.matmul(
                            psv[:, 0:GC],
                            lhsT=xf[kt][:, 128 * m : 128 * (m + 1)],
                            rhs=wt[kt][:, 2 * GC : 3 * GC],
                            start=(kt == 0),
                            stop=(kt == CT - 1),
                        )
                    v3 = vsb[m][:].rearrange("p (h w) -> p h w", h=HG)
                    nc.vector.memset(v3[:, :, 64:65], 1.0)
                    nc.vector.tensor_copy(
                        v3[:, :, 0:64],
                        psv[:, 0:GC].rearrange("p (h w) -> p h w", h=HG),
                    )

                # ---- phase1: K/Q for both pairs; V mostly woven into block 0 ----
                for t in range(2):
                    for which, col, dst in (
                        ("k", GC + 128 * t, ksb[t]),
                        ("q", 128 * t, qsb[t]),
                    ):
                        for (n0, nl) in [(0, 1024), (1024, 1024), (2048, 256)]:
                            ps = spool.tile([128, 1024], FP32, name="pkq", tag="s")
                            for kt in range(CT):
                                for c0 in range(0, nl, 512):
                                    cl = min(512, nl - c0)
                                    nc.tensor.matmul(
                                        ps[:, c0 : c0 + cl],
                                        lhsT=wt[kt][:, col : col + 128],
                                        rhs=xf[kt][:, n0 + c0 : n0 + c0 + cl],
                                        start=(kt == 0),
                                        stop=(kt == CT - 1),
                                    )
                            nc.vector.tensor_copy(dst[:, n0 : n0 + nl], ps[:, 0:nl])
                for m in range(3):
                    v_group(m)

                # ---- phase2 helpers ----
                def normalize_quadrant(t, half, c0, cl, av, q0):
                    """recip from PSUM row 64 of av[:, q0:q0+cl]; bcast; normalize oa."""
                    with nc.allow_low_precision(reason="softmax recip fp16"):
                        nc.vector.reciprocal(
                            rec[t][half][:, c0 : c0 + cl], av[64:65, q0 : q0 + cl]
                        )
                    bc = flex.tile([128, 512], FP32, name="bc", tag="flex")
                    nc.tensor.matmul(
                        bc[0:64, 0:cl], lhsT=ones1[:],
                        rhs=rec[t][half][:, c0 : c0 + cl],
                        start=True, stop=True,
                    )
                    nc.vector.tensor_mul(
                        oa[t][64 * half : 64 * half + 64, c0 : c0 + cl],
                        avsb[t][64 * half : 64 * half + 64, c0 : c0 + cl],
                        bc[0:64, 0:cl],
                    )

                # ---- 8 main blocks ----
                for t in range(2):
                    for c0 in range(0, NT, 512):
                        avA = avps.tile([65, 512], FP32, name="avA", tag="avA")
                        avB = avps.tile([65, 512], FP32, name="avB", tag="avB")
                        for m in range(MT):
                            if t == 0 and c0 == 0 and m + 3 < MT:
                                v_group(m + 3)
                            ms = slice(128 * m, 128 * (m + 1))
                            sp = spool.tile([128, 1024], FP32, name="s", tag="s")
                            nc.tensor.matmul(
                                sp[:, 0:512], lhsT=ksb[t][0:64, ms],
                                rhs=qsb[t][0:64, c0 : c0 + 512],
                                start=True, stop=True, tile_position=(0, 0),
                            )
                            nc.tensor.matmul(
                                sp[:, 512:1024], lhsT=ksb[t][64:128, ms],
                                rhs=qsb[t][64:128, c0 : c0 + 512],
                                start=True, stop=True, tile_position=(64, 0),
                            )
                            es = epool.tile([128, 1024], FP16, name="es", tag="es")
                            nc.scalar.activation(es[:], sp[:], AF.Exp, scale=SCALE)
                            nc.tensor.matmul(
                                avA[:], lhsT=vsb[m][:, 130 * t : 130 * t + 65],
                                rhs=es[:, 0:512],
                                start=(m == 0), stop=(m == MT - 1),
                            )
                            nc.tensor.matmul(
                                avB[:], lhsT=vsb[m][:, 130 * t + 65 : 130 * t + 130],
                                rhs=es[:, 512:1024],
                                start=(m == 0), stop=(m == MT - 1),
                            )
                        nc.vector.tensor_copy(avsb[t][0:64, c0 : c0 + 512], avA[0:64, :])
                        nc.vector.tensor_copy(avsb[t][64:128, c0 : c0 + 512], avB[0:64, :])
                        normalize_quadrant(t, 0, c0, 512, avA, 0)
                        normalize_quadrant(t, 1, c0, 512, avB, 0)
                        for ct in range(CT):
                            py = flex.tile([128, 512], FP32, name="py", tag="flex")
                            nc.tensor.matmul(
                                py[:],
                                lhsT=wp[t][:, 128 * ct : 128 * (ct + 1)],
                                rhs=oa[t][:, c0 : c0 + 512],
                                start=True, stop=True,
                            )
                            if t == 0:
                                nc.vector.tensor_copy(oy[ct][:, c0 : c0 + 512], py[:])
                            else:
                                nc.vector.tensor_add(
                                    oy[ct][:, c0 : c0 + 512],
                                    oy[ct][:, c0 : c0 + 512],
                                    py[:],
                                )
                        if t == 1 and c0 + 512 == NT:
                            for ct2 in range(CT):
                                eng = nc.sync if ct2 % 2 == 0 else nc.scalar
                                eng.dma_start(
                                    y.ap()[128 * ct2 : 128 * (ct2 + 1), 0:NT],
                                    oy[ct2][:, 0:NT],
                                )

                # ---- joint tail block: last 256 queries of both pairs ----
                # sp quadrants: p0A [0:256], p1A [256:512] (row group 0);
                #               p0B [512:768], p1B [768:1024] (row group 64).
                avA = avps.tile([65, 512], FP32, name="avA", tag="avA")
                avB = avps.tile([65, 512], FP32, name="avB", tag="avB")
                av2A = flex.tile([128, 512], FP32, name="av2A", tag="flex")
                av2B = flex.tile([128, 512], FP32, name="av2B", tag="flex")
                for m in range(MT):
                    ms = slice(128 * m, 128 * (m + 1))
                    sp = spool.tile([128, 1024], FP32, name="s", tag="s")
                    nc.tensor.matmul(
                        sp[:, 0:256], lhsT=ksb[0][0:64, ms],
                        rhs=qsb[0][0:64, NT:N],
                        start=True, stop=True, tile_position=(0, 0),
                    )
                    nc.tensor.matmul(
                        sp[:, 256:512], lhsT=ksb[1][0:64, ms],
                        rhs=qsb[1][0:64, NT:N],
                        start=True, stop=True, tile_position=(0, 0),
                    )
                    nc.tensor.matmul(
                        sp[:, 512:768], lhsT=ksb[0][64:128, ms],
                        rhs=qsb[0][64:128, NT:N],
                        start=True, stop=True, tile_position=(64, 0),
                    )
                    nc.tensor.matmul(
                        sp[:, 768:1024], lhsT=ksb[1][64:128, ms],
                        rhs=qsb[1][64:128, NT:N],
                        start=True, stop=True, tile_position=(64, 0),
                    )
                    es = epool.tile([128, 1024], FP16, name="es", tag="es")
                    nc.scalar.activation(es[:], sp[:], AF.Exp, scale=SCALE)
                    nc.tensor.matmul(
                        avA[:, 0:256], lhsT=vsb[m][:, 0:65],
                        rhs=es[:, 0:256],
                        start=(m == 0), stop=(m == MT - 1),
                    )
                    nc.tensor.matmul(
                        av2A[0:65, 0:256], lhsT=vsb[m][:, 130:195],
                        rhs=es[:, 256:512],
                        start=(m == 0), stop=(m == MT - 1),
                    )
                    nc.tensor.matmul(
                        avB[:, 0:256], lhsT=vsb[m][:, 65:130],
                        rhs=es[:, 512:768],
                        start=(m == 0), stop=(m == MT - 1),
                    )
                    nc.tensor.matmul(
                        av2B[0:65, 0:256], lhsT=vsb[m][:, 195:260],
                        rhs=es[:, 768:1024],
                        start=(m == 0), stop=(m == MT - 1),
                    )
                nc.vector.tensor_copy(avsb[0][0:64, NT:N], avA[0:64, 0:256])
                nc.vector.tensor_copy(avsb[1][0:64, NT:N], av2A[0:64, 0:256])
                nc.vector.tensor_copy(avsb[0][64:128, NT:N], avB[0:64, 0:256])
                nc.vector.tensor_copy(avsb[1][64:128, NT:N], av2B[0:64, 0:256])
                normalize_quadrant(0, 0, NT, 256, avA, 0)
                normalize_quadrant(1, 0, NT, 256, av2A, 0)
                normalize_quadrant(0, 1, NT, 256, avB, 0)
                normalize_quadrant(1, 1, NT, 256, av2B, 0)
                for ct in range(CT):
                    py = flex.tile([128, 512], FP32, name="py", tag="flex")
                    for t in range(2):
                        nc.tensor.matmul(
                            py[:, 0:256],
                            lhsT=wp[t][:, 128 * ct : 128 * (ct + 1)],
                            rhs=oa[t][:, NT:N],
                            start=(t == 0), stop=(t == 1),
                        )
                    nc.vector.tensor_copy(oy[ct][:, NT:N], py[:, 0:256])
                    eng = nc.sync if ct % 2 == 0 else nc.scalar
                    eng.dma_start(
                        y.ap()[128 * ct : 128 * (ct + 1), NT:N], oy[ct][:, NT:N]
                    )

            with (
                tc.tile_pool(name="sps", bufs=2, space="PSUM") as spool,
                tc.tile_pool(name="avps", bufs=1, space="PSUM") as avps,
                tc.tile_pool(name="flex", bufs=2, space="PSUM") as flex,
                tc.tile_pool(name="esb", bufs=6) as epool,
            ):
                import contextlib
                loop_ctx = tc.For_i(0, loop_n, 1) if loop_n else contextlib.nullcontext()
                with loop_ctx:
                    _run(spool, avps, flex, epool)

    nc.compile()
    return nc


def _get_module():
    if "nc" not in _CACHE:
        _CACHE["nc"] = _build_module()
    return _CACHE["nc"]


def make_in_maps(x, qkv_w, proj_w):
    xr = np.asarray(x, dtype=np.float32).reshape(B, C, N)
    qkv_w = np.asarray(qkv_w)
    proj_w = np.asarray(proj_w)
    in_maps = []
    for i in range(NCORES):
        b, g = divmod(i, 2)
        rows = np.r_[
            g * GC : (g + 1) * GC,
            C + g * GC : C + (g + 1) * GC,
            2 * C + g * GC : 2 * C + (g + 1) * GC,
        ]
        wq = np.ascontiguousarray(qkv_w[rows, :].T).astype(np.float16)
        wpj = np.ascontiguousarray(proj_w[:, g * GC : (g + 1) * GC].T).astype(np.float16)
        xc = np.ascontiguousarray(xr[b]).astype(np.float16)
        in_maps.append({"xf": xc, "wqkv": wq, "wproj": wpj})
    return in_maps


def gather_out(results):
    out = np.empty((B, C, N), np.float32)
    for b in range(B):
        out[b] = results[2 * b]["y"].astype(np.float32) + results[2 * b + 1]["y"].astype(
            np.float32
        )
    return out.reshape(B, C, HH, WW)


def kernel(x, qkv_w, proj_w):
    from concourse import bass_utils

    nc = _get_module()
    in_maps = make_in_maps(x, qkv_w, proj_w)
    res = bass_utils.run_bass_kernel_spmd(
        nc, in_maps, core_ids=list(range(NCORES)), trace=False
    )
    return gather_out(res.results)


# revision 4
# speedup vs baseline: 1.0162x; 1.0001x over previous
"""Multi-head self-attention 2D Bass kernel v10: v5b + PE warm-up burst during DMA head.

Sharding (8 cores): core i handles batch b = i//2 and HEAD-GROUP g = i%2
  (heads 4g..4g+3), all N=2304 queries.  Per-core: Q/K/V for its 4 heads
  only (no K/V redundancy vs batch-sharding), attention, and a PARTIAL
  projection y_g = Wp[:, g] @ out_g; the host sums the two fp16 partials
  per batch.

Structure (all matmul operands fp16, PSUM fp32, simple in-order
emission -- measured fastest on HW; interleaved/software-pipelined
variants regressed):
  phase1: V (18 m-tiles), then K,Q for pair 0 and pair 1.
  phase2: 8 main blocks (pair t x 512-query chunk c; per m-tile:
    row-packed S pair -> one wide exp [128,1024] (scale folded) ->
    AV pair accumulating with ones-row denominators), then ONE joint
    tail block covering the last 256 queries of BOTH pairs (4 S
    quadrants in bank-separated PSUM, still one wide exp per m).
  Per-block finalize: reciprocal straight from the PSUM denominator
  row, ones-matmul broadcast, normalize, projection slice into oy
  (pair 0 copy / pair 1 add); y cols 0:2048 DMA after the last main
  block, tail cols after the tail block.
"""

import numpy as np

B = 4
C = 512
HH = 48
WW = 48
N = HH * WW          # 2304
HEADS = 8
HG = 4               # heads per core
GC = HG * 64         # 256 channels per group
D = 64
SCALE = float(D) ** -0.5
NCORES = 8

_CACHE: dict = {}


def _build_module(loop_n=None):
    import concourse.mybir as mybir
    import concourse.tile as tile
    from concourse import bacc

    FP16 = mybir.dt.float16
    FP32 = mybir.dt.float32
    AF = mybir.ActivationFunctionType

    nc = bacc.Bacc("TRN2", target_bir_lowering=False, debug=False)
    xf_d = nc.dram_tensor("xf", [C, N], FP16, kind="ExternalInput")
    wqkv = nc.dram_tensor("wqkv", [C, 3 * GC], FP16, kind="ExternalInput")
    wproj = nc.dram_tensor("wproj", [GC, C], FP16, kind="ExternalInput")
    y = nc.dram_tensor("y", [C, N], FP16, kind="ExternalOutput")

    CT = C // 128     # 4 channel tiles of x
    MT = N // 128     # 18 key tiles
    NT = 2048         # tail start

    with tile.TileContext(nc) as tc:
        with (
            tc.tile_pool(name="consts", bufs=1) as cpool,
            tc.tile_pool(name="wts", bufs=1) as wpool,
            tc.tile_pool(name="xin", bufs=1) as xpool,
            tc.tile_pool(name="qkv", bufs=1) as qkpool,
            tc.tile_pool(name="keep", bufs=1) as keep,
        ):
            ones1 = cpool.tile([1, 64], FP16, name="ones1", tag="ones1")
            nc.vector.memset(ones1[:], 1.0)

            wt = []
            for t in range(CT):
                w = wpool.tile([128, 3 * GC], FP16, name=f"w{t}", tag=f"w{t}")
                nc.sync.dma_start(w[:], wqkv.ap()[128 * t : 128 * (t + 1), :])
                wt.append(w)
            wp = []
            for t in range(2):
                p = wpool.tile([128, C], FP16, name=f"wp{t}", tag=f"wp{t}")
                nc.sync.dma_start(p[:], wproj.ap()[128 * t : 128 * (t + 1), :])
                wp.append(p)

            qsb = [qkpool.tile([128, N], FP16, name=f"q{t}", tag=f"q{t}") for t in range(2)]
            ksb = [qkpool.tile([128, N], FP16, name=f"k{t}", tag=f"k{t}") for t in range(2)]
            vsb = [qkpool.tile([128, 4 * 65], FP16, name=f"v{m}", tag=f"v{m}") for m in range(MT)]
            avsb = [keep.tile([128, N], FP16, name=f"av{t}", tag=f"av{t}") for t in range(2)]
            oa = [keep.tile([128, N], FP16, name=f"oa{t}", tag=f"oa{t}") for t in range(2)]
            oy = [keep.tile([128, N], FP16, name=f"oy{ct}", tag=f"oy{ct}") for ct in range(CT)]
            rec = [
                [keep.tile([1, N], FP16, name=f"rec{t}_{h}", tag=f"rec{t}_{h}") for h in range(2)]
                for t in range(2)
            ]

            xf = []
            for t in range(CT):
                xt = xpool.tile([128, N], FP16, name=f"x{t}", tag=f"x{t}")
                nc.scalar.dma_start(
                    xt[:, 0 : N // 2], xf_d.ap()[128 * t : 128 * (t + 1), 0 : N // 2]
                )
                xf.append(xt)
            for t in range(CT):
                nc.scalar.dma_start(
                    xf[t][:, N // 2 : N],
                    xf_d.ap()[128 * t : 128 * (t + 1), N // 2 : N],
                )

            def _run(spool, avps, flex, epool):
                # PE clock-gate warm-up: ~3.5us of tiny matmuls that depend
                # only on the ones constant, issued while the x/w DMAs land,
                # so the HAM un-throttles before the real phase1 matmuls.
                warm = flex.tile([128, 512], FP32, name="warm", tag="flex")
                for _ in range(136):
                    nc.tensor.matmul(
                        warm[0:64, 0:64], lhsT=ones1[:], rhs=ones1[:],
                        start=True, stop=True,
                    )

                def v_group(m):
                    psv = flex.tile([128, 512], FP32, name="psv", tag="flex")
                    for kt in range(CT):
                        nc.tensor.matmul(
                            psv[:, 0:GC],
                            lhsT=xf[kt][:, 128 * m : 128 * (m + 1)],
                            rhs=wt[kt][:, 2 * GC : 3 * GC],
                            start=(kt == 0),
                            stop=(kt == CT - 1),
                        )
                    v3 = vsb[m][:].rearrange("p (h w) -> p h w", h=HG)
                    nc.vector.memset(v3[:, :, 64:65], 1.0)
                    nc.vector.tensor_copy(
                        v3[:, :, 0:64],
                        psv[:, 0:GC].rearrange("p (h w) -> p h w", h=HG),
                    )

                # ---- phase1: K/Q for both pairs; V mostly woven into block 0 ----
                for t in range(2):
                    for which, col, dst in (
                        ("k", GC + 128 * t, ksb[t]),
                        ("q", 128 * t, qsb[t]),
                    ):
                        for (n0, nl) in [(0, 1024), (1024, 1024), (2048, 256)]:
                            ps = spool.tile([128, 1024], FP32, name="pkq", tag="s")
                            for kt in range(CT):
                                for c0 in range(0, nl, 512):
                                    cl = min(512, nl - c0)
                                    nc.tensor.matmul(
                                        ps[:, c0 : c0 + cl],
                                        lhsT=wt[kt][:, col : col + 128],
                                        rhs=xf[kt][:, n0 + c0 : n0 + c0 + cl],
                                        start=(kt == 0),
                                        stop=(kt == CT - 1),
                                    )
                            nc.vector.tensor_copy(dst[:, n0 : n0 + nl], ps[:, 0:nl])
                for m in range(3):
                    v_group(m)

                # ---- phase2 helpers ----
                def normalize_quadrant(t, half, c0, cl, av, q0):
                    """recip from PSUM row 64 of av[:, q0:q0+cl]; bcast; normalize oa."""
                    with nc.allow_low_precision(reason="softmax recip fp16"):
                        nc.vector.reciprocal(
                            rec[t][half][:, c0 : c0 + cl], av[64:65, q0 : q0 + cl]
                        )
                    bc = flex.tile([128, 512], FP32, name="bc", tag="flex")
                    nc.tensor.matmul(
                        bc[0:64, 0:cl], lhsT=ones1[:],
                        rhs=rec[t][half][:, c0 : c0 + cl],
                        start=True, stop=True,
                    )
                    nc.vector.tensor_mul(
                        oa[t][64 * half : 64 * half + 64, c0 : c0 + cl],
                        avsb[t][64 * half : 64 * half + 64, c0 : c0 + cl],
                        bc[0:64, 0:cl],
                    )

                # ---- 8 main blocks ----
                for t in range(2):
                    for c0 in range(0, NT, 512):
                        avA = avps.tile([65, 512], FP32, name="avA", tag="avA")
                        avB = avps.tile([65, 512], FP32, name="avB", tag="avB")
                        for m in range(MT):
                            if t == 0 and c0 == 0 and m + 3 < MT:
                                v_group(m + 3)
                            ms = slice(128 * m, 128 * (m + 1))
                            sp = spool.tile([128, 1024], FP32, name="s", tag="s")
                            nc.tensor.matmul(
                                sp[:, 0:512], lhsT=ksb[t][0:64, ms],
                                rhs=qsb[t][0:64, c0 : c0 + 512],
                                start=True, stop=True, tile_position=(0, 0),
                            )
                            nc.tensor.matmul(
                                sp[:, 512:1024], lhsT=ksb[t][64:128, ms],
                                rhs=qsb[t][64:128, c0 : c0 + 512],
                                start=True, stop=True, tile_position=(64, 0),
                            )
                            es = epool.tile([128, 1024], FP16, name="es", tag="es")
                            nc.scalar.activation(es[:], sp[:], AF.Exp, scale=SCALE)
                            nc.tensor.matmul(
                                avA[:], lhsT=vsb[m][:, 130 * t : 130 * t + 65],
                                rhs=es[:, 0:512],
                                start=(m == 0), stop=(m == MT - 1),
                            )
                            nc.tensor.matmul(
                                avB[:], lhsT=vsb[m][:, 130 * t + 65 : 130 * t + 130],
                                rhs=es[:, 512:1024],
                                start=(m == 0), stop=(m == MT - 1),
                            )
                        nc.vector.tensor_copy(avsb[t][0:64, c0 : c0 + 512], avA[0:64, :])
                        nc.vector.tensor_copy(avsb[t][64:128, c0 : c0 + 512], avB[0:64, :])
                        normalize_quadrant(t, 0, c0, 512, avA, 0)
                        normalize_quadrant(t, 1, c0, 512, avB, 0)
                        for ct in range(CT):
                            py = flex.tile([128, 512], FP32, name="py", tag="flex")
                            nc.tensor.matmul(
                                py[:],
                                lhsT=wp[t][:, 128 * ct : 128 * (ct + 1)],
                                rhs=oa[t][:, c0 : c0 + 512],
                                start=True, stop=True,
                            )
                            if t == 0:
                                nc.vector.tensor_copy(oy[ct][:, c0 : c0 + 512], py[:])
                            else:
                                nc.vector.tensor_add(
                                    oy[ct][:, c0 : c0 + 512],
                                    oy[ct][:, c0 : c0 + 512],
                                    py[:],
                                )
                        if t == 1 and c0 + 512 == NT:
                            for ct2 in range(CT):
                                eng = nc.sync if ct2 % 2 == 0 else nc.scalar
                                eng.dma_start(
                                    y.ap()[128 * ct2 : 128 * (ct2 + 1), 0:NT],
                                    oy[ct2][:, 0:NT],
                                )

                # ---- joint tail block: last 256 queries of both pairs ----
                # sp quadrants: p0A [0:256], p1A [256:512] (row group 0);
                #               p0B [512:768], p1B [768:1024] (row group 64).
                avA = avps.tile([65, 512], FP32, name="avA", tag="avA")
                avB = avps.tile([65, 512], FP32, name="avB", tag="avB")
                av2A = flex.tile([128, 512], FP32, name="av2A", tag="flex")
                av2B = flex.tile([128, 512], FP32, name="av2B", tag="flex")
                for m in range(MT):
                    ms = slice(128 * m, 128 * (m + 1))
                    sp = spool.tile([128, 1024], FP32, name="s", tag="s")
                    nc.tensor.matmul(
                        sp[:, 0:256], lhsT=ksb[0][0:64, ms],
                        rhs=qsb[0][0:64, NT:N],
                        start=True, stop=True, tile_position=(0, 0),
                    )
                    nc.tensor.matmul(
                        sp[:, 256:512], lhsT=ksb[1][0:64, ms],
                        rhs=qsb[1][0:64, NT:N],
                        start=True, stop=True, tile_position=(0, 0),
                    )
                    nc.tensor.matmul(
                        sp[:, 512:768], lhsT=ksb[0][64:128, ms],
                        rhs=qsb[0][64:128, NT:N],
                        start=True, stop=True, tile_position=(64, 0),
                    )
                    nc.tensor.matmul(
                        sp[:, 768:1024], lhsT=ksb[1][64:128, ms],
                        rhs=qsb[1][64:128, NT:N],
                        start=True, stop=True, tile_position=(64, 0),
                    )
                    es = epool.tile([128, 1024], FP16, name="es", tag="es")
                    nc.scalar.activation(es[:], sp[:], AF.Exp, scale=SCALE)
                    nc.tensor.matmul(
                        avA[:, 0:256], lhsT=vsb[m][:, 0:65],
                        rhs=es[:, 0:256],
                        start=(m == 0), stop=(m == MT - 1),
                    )
                    nc.tensor.matmul(
                        av2A[0:65, 0:256], lhsT=vsb[m][:, 130:195],
                        rhs=es[:, 256:512],
                        start=(m == 0), stop=(m == MT - 1),
                    )
                    nc.tensor.matmul(
                        avB[:, 0:256], lhsT=vsb[m][:, 65:130],
                        rhs=es[:, 512:768],
                        start=(m == 0), stop=(m == MT - 1),
                    )
                    nc.tensor.matmul(
                        av2B[0:65, 0:256], lhsT=vsb[m][:, 195:260],
                        rhs=es[:, 768:1024],
                        start=(m == 0), stop=(m == MT - 1),
                    )
                nc.vector.tensor_copy(avsb[0][0:64, NT:N], avA[0:64, 0:256])
                nc.vector.tensor_copy(avsb[1][0:64, NT:N], av2A[0:64, 0:256])
                nc.vector.tensor_copy(avsb[0][64:128, NT:N], avB[0:64, 0:256])
                nc.vector.tensor_copy(avsb[1][64:128, NT:N], av2B[0:64, 0:256])
                normalize_quadrant(0, 0, NT, 256, avA, 0)
                normalize_quadrant(1, 0, NT, 256, av2A, 0)
                normalize_quadrant(0, 1, NT, 256, avB, 0)
                normalize_quadrant(1, 1, NT, 256, av2B, 0)
                for ct in range(CT):
                    py = flex.tile([128, 512], FP32, name="py", tag="flex")
                    for t in range(2):
                        nc.tensor.matmul(
                            py[:, 0:256],
                            lhsT=wp[t][:, 128 * ct : 128 * (ct + 1)],
                            rhs=oa[t][:, NT:N],
                            start=(t == 0), stop=(t == 1),
                        )
                    nc.vector.tensor_copy(oy[ct][:, NT:N], py[:, 0:256])
                    eng = nc.sync if ct % 2 == 0 else nc.scalar
                    eng.dma_start(
                        y.ap()[128 * ct : 128 * (ct + 1), NT:N], oy[ct][:, NT:N]
                    )

            with (
                tc.tile_pool(name="sps", bufs=2, space="PSUM") as spool,
                tc.tile_pool(name="avps", bufs=1, space="PSUM") as avps,
                tc.tile_pool(name="flex", bufs=2, space="PSUM") as flex,
                tc.tile_pool(name="esb", bufs=6) as epool,
            ):
                import contextlib
                loop_ctx = tc.For_i(0, loop_n, 1) if loop_n else contextlib.nullcontext()
                with loop_ctx:
                    _run(spool, avps, flex, epool)

    nc.compile()
    return nc


def _get_module():
    if "nc" not in _CACHE:
        _CACHE["nc"] = _build_module()
    return _CACHE["nc"]


def make_in_maps(x, qkv_w, proj_w):
    xr = np.asarray(x, dtype=np.float32).reshape(B, C, N)
    qkv_w = np.asarray(qkv_w)
    proj_w = np.asarray(proj_w)
    in_maps = []
    for i in range(NCORES):
        b, g = divmod(i, 2)
        rows = np.r_[
            g * GC : (g + 1) * GC,
            C + g * GC : C + (g + 1) * GC,
            2 * C + g * GC : 2 * C + (g + 1) * GC,
        ]
        wq = np.ascontiguousarray(qkv_w[rows, :].T).astype(np.float16)
        wpj = np.ascontiguousarray(proj_w[:, g * GC : (g + 1) * GC].T).astype(np.float16)
        xc = np.ascontiguousarray(xr[b]).astype(np.float16)
        in_maps.append({"xf": xc, "wqkv": wq, "wproj": wpj})
    return in_maps


def gather_out(results):
    out = np.empty((B, C, N), np.float32)
    for b in range(B):
        out[b] = results[2 * b]["y"].astype(np.float32) + results[2 * b + 1]["y"].astype(
            np.float32
        )
    return out.reshape(B, C, HH, WW)


def kernel(x, qkv_w, proj_w):
    from concourse import bass_utils

    nc = _get_module()
    in_maps = make_in_maps(x, qkv_w, proj_w)
    res = bass_utils.run_bass_kernel_spmd(
        nc, in_maps, core_ids=list(range(NCORES)), trace=False
    )
    return gather_out(res.results)


# revision 5
# speedup vs baseline: 1.0165x; 1.0003x over previous
"""Multi-head self-attention 2D Bass kernel v12: v10 + deeper es pipeline.

Sharding (8 cores): core i handles batch b = i//2 and HEAD-GROUP g = i%2
  (heads 4g..4g+3), all N=2304 queries.  Per-core: Q/K/V for its 4 heads
  only (no K/V redundancy vs batch-sharding), attention, and a PARTIAL
  projection y_g = Wp[:, g] @ out_g; the host sums the two fp16 partials
  per batch.

Structure (all matmul operands fp16, PSUM fp32, simple in-order
emission -- measured fastest on HW; interleaved/software-pipelined
variants regressed):
  phase1: V (18 m-tiles), then K,Q for pair 0 and pair 1.
  phase2: 8 main blocks (pair t x 512-query chunk c; per m-tile:
    row-packed S pair -> one wide exp [128,1024] (scale folded) ->
    AV pair accumulating with ones-row denominators), then ONE joint
    tail block covering the last 256 queries of BOTH pairs (4 S
    quadrants in bank-separated PSUM, still one wide exp per m).
  Per-block finalize: reciprocal straight from the PSUM denominator
  row, ones-matmul broadcast, normalize, projection slice into oy
  (pair 0 copy / pair 1 add); y cols 0:2048 DMA after the last main
  block, tail cols after the tail block.
"""

import numpy as np

B = 4
C = 512
HH = 48
WW = 48
N = HH * WW          # 2304
HEADS = 8
HG = 4               # heads per core
GC = HG * 64         # 256 channels per group
D = 64
SCALE = float(D) ** -0.5
NCORES = 8

_CACHE: dict = {}


def _build_module(loop_n=None):
    import concourse.mybir as mybir
    import concourse.tile as tile
    from concourse import bacc

    FP16 = mybir.dt.float16
    FP32 = mybir.dt.float32
    AF = mybir.ActivationFunctionType

    nc = bacc.Bacc("TRN2", target_bir_lowering=False, debug=False)
    xf_d = nc.dram_tensor("xf", [C, N], FP16, kind="ExternalInput")
    wqkv = nc.dram_tensor("wqkv", [C, 3 * GC], FP16, kind="ExternalInput")
    wproj = nc.dram_tensor("wproj", [GC, C], FP16, kind="ExternalInput")
    y = nc.dram_tensor("y", [C, N], FP16, kind="ExternalOutput")

    CT = C // 128     # 4 channel tiles of x
    MT = N // 128     # 18 key tiles
    NT = 2048         # tail start

    with tile.TileContext(nc) as tc:
        with (
            tc.tile_pool(name="consts", bufs=1) as cpool,
            tc.tile_pool(name="wts", bufs=1) as wpool,
            tc.tile_pool(name="xin", bufs=1) as xpool,
            tc.tile_pool(name="qkv", bufs=1) as qkpool,
            tc.tile_pool(name="keep", bufs=1) as keep,
        ):
            ones1 = cpool.tile([1, 64], FP16, name="ones1", tag="ones1")
            nc.vector.memset(ones1[:], 1.0)

            wt = []
            for t in range(CT):
                w = wpool.tile([128, 3 * GC], FP16, name=f"w{t}", tag=f"w{t}")
                nc.sync.dma_start(w[:], wqkv.ap()[128 * t : 128 * (t + 1), :])
                wt.append(w)
            wp = []
            for t in range(2):
                p = wpool.tile([128, C], FP16, name=f"wp{t}", tag=f"wp{t}")
                nc.sync.dma_start(p[:], wproj.ap()[128 * t : 128 * (t + 1), :])
                wp.append(p)

            qsb = [qkpool.tile([128, N], FP16, name=f"q{t}", tag=f"q{t}") for t in range(2)]
            ksb = [qkpool.tile([128, N], FP16, name=f"k{t}", tag=f"k{t}") for t in range(2)]
            vsb = [qkpool.tile([128, 4 * 65], FP16, name=f"v{m}", tag=f"v{m}") for m in range(MT)]
            avsb = [keep.tile([128, N], FP16, name=f"av{t}", tag=f"av{t}") for t in range(2)]
            oa = [keep.tile([128, N], FP16, name=f"oa{t}", tag=f"oa{t}") for t in range(2)]
            oy = [keep.tile([128, N], FP16, name=f"oy{ct}", tag=f"oy{ct}") for ct in range(CT)]
            rec = [
                [keep.tile([1, N], FP16, name=f"rec{t}_{h}", tag=f"rec{t}_{h}") for h in range(2)]
                for t in range(2)
            ]

            xf = []
            for t in range(CT):
                xt = xpool.tile([128, N], FP16, name=f"x{t}", tag=f"x{t}")
                nc.scalar.dma_start(
                    xt[:, 0 : N // 2], xf_d.ap()[128 * t : 128 * (t + 1), 0 : N // 2]
                )
                xf.append(xt)
            for t in range(CT):
                nc.scalar.dma_start(
                    xf[t][:, N // 2 : N],
                    xf_d.ap()[128 * t : 128 * (t + 1), N // 2 : N],
                )

            def _run(spool, avps, flex, epool):
                # PE clock-gate warm-up: ~3.5us of tiny matmuls that depend
                # only on the ones constant, issued while the x/w DMAs land,
                # so the HAM un-throttles before the real phase1 matmuls.
                warm = flex.tile([128, 512], FP32, name="warm", tag="flex")
                for _ in range(136):
                    nc.tensor.matmul(
                        warm[0:64, 0:64], lhsT=ones1[:], rhs=ones1[:],
                        start=True, stop=True,
                    )

                def v_group(m):
                    psv = flex.tile([128, 512], FP32, name="psv", tag="flex")
                    for kt in range(CT):
                        nc.tensor.matmul(
                            psv[:, 0:GC],
                            lhsT=xf[kt][:, 128 * m : 128 * (m + 1)],
                            rhs=wt[kt][:, 2 * GC : 3 * GC],
                            start=(kt == 0),
                            stop=(kt == CT - 1),
                        )
                    v3 = vsb[m][:].rearrange("p (h w) -> p h w", h=HG)
                    nc.vector.memset(v3[:, :, 64:65], 1.0)
                    nc.vector.tensor_copy(
                        v3[:, :, 0:64],
                        psv[:, 0:GC].rearrange("p (h w) -> p h w", h=HG),
                    )

                # ---- phase1: K/Q for both pairs; V mostly woven into block 0 ----
                for t in range(2):
                    for which, col, dst in (
                        ("k", GC + 128 * t, ksb[t]),
                        ("q", 128 * t, qsb[t]),
                    ):
                        for (n0, nl) in [(0, 1024), (1024, 1024), (2048, 256)]:
                            ps = spool.tile([128, 1024], FP32, name="pkq", tag="s")
                            for kt in range(CT):
                                for c0 in range(0, nl, 512):
                                    cl = min(512, nl - c0)
                                    nc.tensor.matmul(
                                        ps[:, c0 : c0 + cl],
                                        lhsT=wt[kt][:, col : col + 128],
                                        rhs=xf[kt][:, n0 + c0 : n0 + c0 + cl],
                                        start=(kt == 0),
                                        stop=(kt == CT - 1),
                                    )
                            nc.vector.tensor_copy(dst[:, n0 : n0 + nl], ps[:, 0:nl])
                for m in range(3):
                    v_group(m)

                # ---- phase2 helpers ----
                def normalize_quadrant(t, half, c0, cl, av, q0):
                    """recip from PSUM row 64 of av[:, q0:q0+cl]; bcast; normalize oa."""
                    with nc.allow_low_precision(reason="softmax recip fp16"):
                        nc.vector.reciprocal(
                            rec[t][half][:, c0 : c0 + cl], av[64:65, q0 : q0 + cl]
                        )
                    bc = flex.tile([128, 512], FP32, name="bc", tag="flex")
                    nc.tensor.matmul(
                        bc[0:64, 0:cl], lhsT=ones1[:],
                        rhs=rec[t][half][:, c0 : c0 + cl],
                        start=True, stop=True,
                    )
                    nc.vector.tensor_mul(
                        oa[t][64 * half : 64 * half + 64, c0 : c0 + cl],
                        avsb[t][64 * half : 64 * half + 64, c0 : c0 + cl],
                        bc[0:64, 0:cl],
                    )

                # ---- 8 main blocks ----
                for t in range(2):
                    for c0 in range(0, NT, 512):
                        avA = avps.tile([65, 512], FP32, name="avA", tag="avA")
                        avB = avps.tile([65, 512], FP32, name="avB", tag="avB")
                        for m in range(MT):
                            if t == 0 and c0 == 0 and m + 3 < MT:
                                v_group(m + 3)
                            ms = slice(128 * m, 128 * (m + 1))
                            sp = spool.tile([128, 1024], FP32, name="s", tag="s")
                            nc.tensor.matmul(
                                sp[:, 0:512], lhsT=ksb[t][0:64, ms],
                                rhs=qsb[t][0:64, c0 : c0 + 512],
                                start=True, stop=True, tile_position=(0, 0),
                            )
                            nc.tensor.matmul(
                                sp[:, 512:1024], lhsT=ksb[t][64:128, ms],
                                rhs=qsb[t][64:128, c0 : c0 + 512],
                                start=True, stop=True, tile_position=(64, 0),
                            )
                            es = epool.tile([128, 1024], FP16, name="es", tag="es")
                            nc.scalar.activation(es[:], sp[:], AF.Exp, scale=SCALE)
                            nc.tensor.matmul(
                                avA[:], lhsT=vsb[m][:, 130 * t : 130 * t + 65],
                                rhs=es[:, 0:512],
                                start=(m == 0), stop=(m == MT - 1),
                            )
                            nc.tensor.matmul(
                                avB[:], lhsT=vsb[m][:, 130 * t + 65 : 130 * t + 130],
                                rhs=es[:, 512:1024],
                                start=(m == 0), stop=(m == MT - 1),
                            )
                        nc.vector.tensor_copy(avsb[t][0:64, c0 : c0 + 512], avA[0:64, :])
                        nc.vector.tensor_copy(avsb[t][64:128, c0 : c0 + 512], avB[0:64, :])
                        normalize_quadrant(t, 0, c0, 512, avA, 0)
                        normalize_quadrant(t, 1, c0, 512, avB, 0)
                        for ct in range(CT):
                            py = flex.tile([128, 512], FP32, name="py", tag="flex")
                            nc.tensor.matmul(
                                py[:],
                                lhsT=wp[t][:, 128 * ct : 128 * (ct + 1)],
                                rhs=oa[t][:, c0 : c0 + 512],
                                start=True, stop=True,
                            )
                            if t == 0:
                                nc.vector.tensor_copy(oy[ct][:, c0 : c0 + 512], py[:])
                            else:
                                nc.vector.tensor_add(
                                    oy[ct][:, c0 : c0 + 512],
                                    oy[ct][:, c0 : c0 + 512],
                                    py[:],
                                )
                        if t == 1 and c0 + 512 == NT:
                            for ct2 in range(CT):
                                eng = nc.sync if ct2 % 2 == 0 else nc.scalar
                                eng.dma_start(
                                    y.ap()[128 * ct2 : 128 * (ct2 + 1), 0:NT],
                                    oy[ct2][:, 0:NT],
                                )

                # ---- joint tail block: last 256 queries of both pairs ----
                # sp quadrants: p0A [0:256], p1A [256:512] (row group 0);
                #               p0B [512:768], p1B [768:1024] (row group 64).
                avA = avps.tile([65, 512], FP32, name="avA", tag="avA")
                avB = avps.tile([65, 512], FP32, name="avB", tag="avB")
                av2A = flex.tile([128, 512], FP32, name="av2A", tag="flex")
                av2B = flex.tile([128, 512], FP32, name="av2B", tag="flex")
                for m in range(MT):
                    ms = slice(128 * m, 128 * (m + 1))
                    sp = spool.tile([128, 1024], FP32, name="s", tag="s")
                    nc.tensor.matmul(
                        sp[:, 0:256], lhsT=ksb[0][0:64, ms],
                        rhs=qsb[0][0:64, NT:N],
                        start=True, stop=True, tile_position=(0, 0),
                    )
                    nc.tensor.matmul(
                        sp[:, 256:512], lhsT=ksb[1][0:64, ms],
                        rhs=qsb[1][0:64, NT:N],
                        start=True, stop=True, tile_position=(0, 0),
                    )
                    nc.tensor.matmul(
                        sp[:, 512:768], lhsT=ksb[0][64:128, ms],
                        rhs=qsb[0][64:128, NT:N],
                        start=True, stop=True, tile_position=(64, 0),
                    )
                    nc.tensor.matmul(
                        sp[:, 768:1024], lhsT=ksb[1][64:128, ms],
                        rhs=qsb[1][64:128, NT:N],
                        start=True, stop=True, tile_position=(64, 0),
                    )
                    es = epool.tile([128, 1024], FP16, name="es", tag="es")
                    nc.scalar.activation(es[:], sp[:], AF.Exp, scale=SCALE)
                    nc.tensor.matmul(
                        avA[:, 0:256], lhsT=vsb[m][:, 0:65],
                        rhs=es[:, 0:256],
                        start=(m == 0), stop=(m == MT - 1),
                    )
                    nc.tensor.matmul(
                        av2A[0:65, 0:256], lhsT=vsb[m][:, 130:195],
                        rhs=es[:, 256:512],
                        start=(m == 0), stop=(m == MT - 1),
                    )
                    nc.tensor.matmul(
                        avB[:, 0:256], lhsT=vsb[m][:, 65:130],
                        rhs=es[:, 512:768],
                        start=(m == 0), stop=(m == MT - 1),
                    )
                    nc.tensor.matmul(
                        av2B[0:65, 0:256], lhsT=vsb[m][:, 195:260],
                        rhs=es[:, 768:1024],
                        start=(m == 0), stop=(m == MT - 1),
                    )
                nc.vector.tensor_copy(avsb[0][0:64, NT:N], avA[0:64, 0:256])
                nc.vector.tensor_copy(avsb[1][0:64, NT:N], av2A[0:64, 0:256])
                nc.vector.tensor_copy(avsb[0][64:128, NT:N], avB[0:64, 0:256])
                nc.vector.tensor_copy(avsb[1][64:128, NT:N], av2B[0:64, 0:256])
                normalize_quadrant(0, 0, NT, 256, avA, 0)
                normalize_quadrant(1, 0, NT, 256, av2A, 0)
                normalize_quadrant(0, 1, NT, 256, avB, 0)
                normalize_quadrant(1, 1, NT, 256, av2B, 0)
                for ct in range(CT):
                    py = flex.tile([128, 512], FP32, name="py", tag="flex")
                    for t in range(2):
                        nc.tensor.matmul(
                            py[:, 0:256],
                            lhsT=wp[t][:, 128 * ct : 128 * (ct + 1)],
                            rhs=oa[t][:, NT:N],
                            start=(t == 0), stop=(t == 1),
                        )
                    nc.vector.tensor_copy(oy[ct][:, NT:N], py[:, 0:256])
                    eng = nc.sync if ct % 2 == 0 else nc.scalar
                    eng.dma_start(
                        y.ap()[128 * ct : 128 * (ct + 1), NT:N], oy[ct][:, NT:N]
                    )

            with (
                tc.tile_pool(name="sps", bufs=2, space="PSUM") as spool,
                tc.tile_pool(name="avps", bufs=1, space="PSUM") as avps,
                tc.tile_pool(name="flex", bufs=2, space="PSUM") as flex,
                tc.tile_pool(name="esb", bufs=8) as epool,
            ):
                import contextlib
                loop_ctx = tc.For_i(0, loop_n, 1) if loop_n else contextlib.nullcontext()
                with loop_ctx:
                    _run(spool, avps, flex, epool)

    nc.compile()
    return nc


def _get_module():
    if "nc" not in _CACHE:
        _CACHE["nc"] = _build_module()
    return _CACHE["nc"]


def make_in_maps(x, qkv_w, proj_w):
    xr = np.asarray(x, dtype=np.float32).reshape(B, C, N)
    qkv_w = np.asarray(qkv_w)
    proj_w = np.asarray(proj_w)
    in_maps = []
    for i in range(NCORES):
        b, g = divmod(i, 2)
        rows = np.r_[
            g * GC : (g + 1) * GC,
            C + g * GC : C + (g + 1) * GC,
            2 * C + g * GC : 2 * C + (g + 1) * GC,
        ]
        wq = np.ascontiguousarray(qkv_w[rows, :].T).astype(np.float16)
        wpj = np.ascontiguousarray(proj_w[:, g * GC : (g + 1) * GC].T).astype(np.float16)
        xc = np.ascontiguousarray(xr[b]).astype(np.float16)
        in_maps.append({"xf": xc, "wqkv": wq, "wproj": wpj})
    return in_maps


def gather_out(results):
    out = np.empty((B, C, N), np.float32)
    for b in range(B):
        out[b] = results[2 * b]["y"].astype(np.float32) + results[2 * b + 1]["y"].astype(
            np.float32
        )
    return out.reshape(B, C, HH, WW)


def kernel(x, qkv_w, proj_w):
    from concourse import bass_utils

    nc = _get_module()
    in_maps = make_in_maps(x, qkv_w, proj_w)
    res = bass_utils.run_bass_kernel_spmd(
        nc, in_maps, core_ids=list(range(NCORES)), trace=False
    )
    return gather_out(res.results)
